# revision 21
# baseline (speedup 1.0000x reference)
"""Trainium2 Bass kernel for the soft Bezier rasterizer (nn_DiffRasterizer).

Contract: kernel(**inputs) takes FULL unsharded inputs (as produced by
reference.setup_inputs()) and returns the FULL (384,384,3) float32 image.

Strategy (pixel-spatial sharding, zero cross-core communication):
  * 8 shapes x 30 polyline segments; image 384x384. Core i owns rows
    [48*i, 48*i+48). Every per-(pixel,segment) quantity the rasterizer
    needs is a quadratic in px along an image row, so the host bakes,
    for every (row, col-block), a (3, 960) weight matrix over the pixel
    feature vector [dx^2, dx, 1] (dx re-centered per 128-col block for
    fp32 accuracy). The PE evaluates all segments for 128 pixels at
    once; ACT/DVE/GPSIMD do the nonlinear tail:
      blocks: R' = sqrt(1e9)*(t*-0.5) | E = perp dist^2 | D2 = vertex
              dist^2 | C2 = 1e25 * sigma^2 * cross
      seg candidate  = E + relu(R'^2 - 0.25e9)   (huge iff t* outside [0,1])
      vertex candidate = D2 ;  min over both sets == exact min dist^2
      winding: ssum = sum sign(C2); inside <=> ssum != -sum(sigma)
      cov = sigmoid(-sign*sqrt(min+eps)/0.01)
  * composite: premultiplied alpha over 8 z-sorted layers (csg all
    false for the graded inputs; a numpy fallback handles csg!=0).
"""
import sys
import os
import numpy as np

for _p in ('/opt/trn_rl_repo',):
    if _p not in sys.path and os.path.isdir(_p):
        sys.path.insert(0, _p)

N = 8            # shapes
S = 30           # polyline samples per shape
HW = 384         # image height == width
EPS = 1e-8
RSC = float(np.sqrt(1e9))   # R' scale
PEN = np.float32(0.25e9)    # relu threshold = 0.25*RSC^2
CSC = 1e25                  # cross scale (sign saturation)
NCORES = 8
RPC = HW // NCORES          # rows per core = 48
CB = 3                      # 128-wide col blocks per row
NT = RPC * CB               # pixel tiles per core = 144
NSEG = N * S                # 240
W_COLS = 4 * NSEG           # [R' | E | D2 | C2] = 960


# ---------------------------------------------------------------- host math
def _bezier_to_polyline(cp, n_samples=S):
    t_global = np.linspace(0.0, 4.0 - 4.0 / n_samples, n_samples)
    seg = np.clip(np.floor(t_global).astype(np.int64), 0, 3)
    t = t_global - seg
    ti = 1.0 - t
    basis = np.stack([ti**3, 3*ti**2*t, 3*ti*t**2, t**3], axis=-1)
    idx = np.stack([seg*3, seg*3+1, seg*3+2, (seg*3+3) % 12], axis=-1)
    gathered = cp[:, idx, :]
    return np.einsum('sk,mskd->msd', basis, gathered)


def _precompute(P, c, alpha, alive, z, csg):
    P = np.asarray(P, np.float64)
    sig_alive = 1.0 / (1.0 + np.exp(-np.asarray(alive, np.float64)))
    active = sig_alive > 0.1
    eff_alpha = np.where(active, np.asarray(alpha, np.float64), 0.0)
    order = np.argsort(np.asarray(z, np.float64), kind='stable')
    P_s = P[order]
    c_s = np.asarray(c, np.float64)[order]
    a_s = eff_alpha[order]
    csg_s = np.asarray(csg, bool)[order]

    poly = _bezier_to_polyline(P_s).astype(np.float32).astype(np.float64)
    a = poly
    b = np.roll(poly, -1, axis=1)
    ax, ay = a[..., 0].ravel(), a[..., 1].ravel()
    bx, by = b[..., 0].ravel(), b[..., 1].ravel()
    abx, aby = bx - ax, by - ay
    ab2e = abx**2 + aby**2 + EPS
    inv = 1.0 / ab2e

    y = np.linspace(0.0, 1.0, HW)
    x = np.linspace(0.0, 1.0, HW)

    py = y[:, None]                                 # (384,1) vs (240,)
    e = aby*py - abx*ax - aby*ay                    # D1 = abx*px + e
    Rw = np.stack([np.zeros_like(e),
                   np.broadcast_to(abx*inv, e.shape),
                   e*inv - 0.5], axis=1) * RSC
    Ew = np.stack([np.broadcast_to(1.0 - abx**2*inv, e.shape),
                   -2*ax - 2*abx*e*inv,
                   ax**2 + (py - ay)**2 - e**2*inv], axis=1)
    Dw = np.stack([np.ones_like(e),
                   np.broadcast_to(-2*ax, e.shape),
                   ax**2 + (py - ay)**2], axis=1)
    up = (ay <= py) & (py < by)
    dn = (ay > py) & (py >= by)
    sigma = np.where(up, 1.0, 0.0) - np.where(dn, 1.0, 0.0)
    s2 = sigma**2
    Cw = np.stack([np.zeros_like(e),
                   np.broadcast_to(-aby, e.shape) * s2 * CSC,
                   (abx*(py - ay) + ax*aby) * s2 * CSC], axis=1)
    ck = (-sigma.reshape(HW, N, S).sum(-1)).astype(np.float32)   # (384,8)

    Wq = np.concatenate([Rw, Ew, Dw, Cw], axis=2)   # (384,3,960) f64

    px0 = np.array([x[cb*128:(cb+1)*128].mean() for cb in range(CB)])
    A, B, C = Wq[:, 0], Wq[:, 1], Wq[:, 2]
    Wfull = np.empty((HW, CB, 3, W_COLS), np.float32)
    for cb in range(CB):
        p0 = px0[cb]
        Wfull[:, cb, 0] = A
        Wfull[:, cb, 1] = 2*A*p0 + B
        Wfull[:, cb, 2] = A*p0*p0 + B*p0 + C

    dxf = x - np.repeat(px0, 128)
    xfeat = np.stack([dxf**2, dxf, np.ones_like(dxf)], 0).astype(np.float32)

    # ck replicated across partitions: (128, 48, 8) per core
    return dict(Wfull=Wfull, ck=ck, xfeat=xfeat,
                c_s=c_s.astype(np.float32), a_s=a_s.astype(np.float32),
                csg_s=csg_s, poly=poly.astype(np.float32))


# ------------------------------------------------------------- bass program
def _build_program():
    import concourse.bass as bass
    import concourse.bacc as bacc
    import concourse.mybir as mybir
    from concourse import tile

    dt = mybir.dt.float32
    AF = mybir.ActivationFunctionType
    ALU = mybir.AluOpType
    AX = mybir.AxisListType

    nc = bacc.Bacc()
    w_d = nc.declare_dram_parameter("w", [RPC, 3, CB, W_COLS], dt, isOutput=False)
    xf_d = nc.declare_dram_parameter("xfeat", [3, CB, 128], dt, isOutput=False)
    ck_d = nc.declare_dram_parameter("ck", [128, RPC * N], dt, isOutput=False)
    ident_d = nc.declare_dram_parameter("ident", [128, 128], dt, isOutput=False)
    cst_d = nc.declare_dram_parameter("consts", [128, 34], dt, isOutput=False)
    out_d = nc.declare_dram_parameter("out", [3, NT, 128], dt, isOutput=True)

    with tile.TileContext(nc) as tc:
        with (
            tc.tile_pool(name="const", bufs=1) as cpool,
            tc.tile_pool(name="wpool", bufs=3) as wpool,
            tc.tile_pool(name="work", bufs=4) as work,
            tc.tile_pool(name="slabs", bufs=1) as slabs,
            tc.tile_pool(name="small", bufs=6) as small,
            tc.tile_pool(name="psA", bufs=3, space=bass.MemorySpace.PSUM) as psa,
            tc.tile_pool(name="psB", bufs=3, space=bass.MemorySpace.PSUM) as psb,
            tc.tile_pool(name="psT", bufs=1, space=bass.MemorySpace.PSUM) as pst,
        ):
            xfeat = cpool.tile([3, CB, 128], dt)
            nc.sync.dma_start(xfeat[:], xf_d[:])
            ckt = cpool.tile([128, RPC * N], dt)
            nc.sync.dma_start(ckt[:], ck_d[:])
            ident = cpool.tile([128, 128], dt)
            nc.sync.dma_start(ident[:], ident_d[:])
            cst = cpool.tile([128, 34], dt)
            nc.sync.dma_start(cst[:], cst_d[:])
            c_eps = cst[:, 0:1]
            c_m100 = cst[:, 1:2]
            c_nalpha = lambda s: cst[:, 2+s:3+s]
            c_acol = lambda s, ch: cst[:, 10+s*3+ch:11+s*3+ch]

            la_all = slabs.tile([128, N, NT], dt)     # coverage*... cov layers

            for r in range(RPC):
                wt = wpool.tile([3, CB, W_COLS], dt, tag="w")
                nc.sync.dma_start(wt[:], w_d[r])
                for cb in range(CB):
                    t = r * CB + cb
                    pA = psa.tile([128, 480], dt, tag="pA")
                    pB = psb.tile([128, 480], dt, tag="pB")
                    # [R' | E] and [D2 | C2]
                    nc.tensor.matmul(pA[:], xfeat[:, cb, :], wt[:, cb, 0:480],
                                     start=True, stop=True)
                    nc.tensor.matmul(pB[:], xfeat[:, cb, :],
                                     wt[:, cb, 480:960], start=True, stop=True)
                    r2 = work.tile([128, NSEG], dt, tag="r2")
                    nc.scalar.activation(r2[:], pA[:, 0:NSEG], AF.Square)
                    pen = work.tile([128, NSEG], dt, tag="pen")
                    # relu(r2 - PEN) on gpsimd
                    nc.gpsimd.tensor_scalar(pen[:], r2[:], -float(PEN), 0.0,
                                            ALU.add, ALU.max)
                    slab = work.tile([128, 2, NSEG], dt, tag="slab")
                    # seg candidates = E + pen
                    nc.vector.tensor_tensor(slab[:, 0, :], pA[:, NSEG:2*NSEG],
                                            pen[:], ALU.add)
                    # vertex candidates = D2
                    nc.scalar.activation(slab[:, 1, :], pB[:, 0:NSEG], AF.Copy)
                    s1 = work.tile([128, NSEG], dt, tag="s1")
                    nc.scalar.activation(s1[:], pB[:, NSEG:2*NSEG], AF.Sign)

                    # reduce: min over (g=2, e=30) per shape -> (128, 8)
                    mind = small.tile([128, N], dt, tag="mind")
                    cand = slab[:].rearrange("p g (n s) -> p n g s", n=N)
                    nc.vector.tensor_reduce(mind[:], cand, AX.XY, ALU.min)
                    ssum = small.tile([128, N], dt, tag="ssum")
                    nc.vector.tensor_reduce(
                        ssum[:], s1[:].rearrange("p (n s) -> p n s", n=N),
                        AX.X, ALU.add)

                    m0 = small.tile([128, N], dt, tag="m0")
                    nc.gpsimd.tensor_scalar_max(m0[:], mind[:], 0.0)
                    sd = small.tile([128, N], dt, tag="sd")
                    nc.scalar.activation(sd[:], m0[:], AF.Sqrt, bias=c_eps)
                    ins = small.tile([128, N], dt, tag="ins")
                    nc.vector.tensor_tensor(ins[:], ssum[:],
                                            ckt[:, r*N:(r+1)*N], ALU.not_equal)
                    sgn = small.tile([128, N], dt, tag="sgn")
                    nc.gpsimd.tensor_scalar(sgn[:], ins[:], -2.0, 1.0,
                                            ALU.mult, ALU.add)
                    sdf = small.tile([128, N], dt, tag="sdf")
                    nc.vector.tensor_tensor(sdf[:], sgn[:], sd[:], ALU.mult)
                    # cov = sigmoid(-100*sdf) written straight into la_all[:, :, t]
                    nc.scalar.activation(la_all[:, :, t], sdf[:],
                                         AF.Sigmoid, scale=-100.0)

            # ---- composite: prgb' = prgb + (alpha_s*cov)*(col_ch - prgb)
            prgb = slabs.tile([128, 3, NT], dt)
            nc.gpsimd.memset(prgb[:], 0.0)
            for s in range(N):
                la_s = la_all[:, s, :]
                for ch in range(3):
                    diff = work.tile([128, NT], dt, tag="diff")
                    # diff = -alpha*prgb + alpha*col
                    nc.scalar.activation(diff[:], prgb[:, ch, :], AF.Copy,
                                         scale=float(-ALPHA_S[s]),
                                         bias=float(ALPHA_S[s] * COL_S[s][ch]))
                    m = work.tile([128, NT], dt, tag="m")
                    nc.vector.tensor_tensor(m[:], la_s, diff[:], ALU.mult)
                    nc.vector.tensor_tensor(prgb[:, ch, :], prgb[:, ch, :],
                                            m[:], ALU.add)
            for ch in range(3):
                nc.gpsimd.tensor_scalar(prgb[:, ch, :], prgb[:, ch, :],
                                        0.0, 1.0, ALU.max, ALU.min)

            # ---- output transpose: (128 p, 144 t) -> (144 t, 128 p) per ch
            for ch in range(3):
                t1 = pst.tile([128, 128], dt, tag="t1")
                nc.tensor.transpose(t1[:], prgb[:, ch, 0:128], ident[:])
                o1 = work.tile([128, 128], dt, tag="o1")
                nc.vector.tensor_copy(o1[:], t1[:])
                nc.sync.dma_start(out_d[ch, 0:128, :], o1[:])
                t2 = pst.tile([16, 128], dt, tag="t2")
                nc.tensor.transpose(t2[:], prgb[:, ch, 128:NT], ident[:])
                o2 = work.tile([16, 128], dt, tag="o2")
                nc.vector.tensor_copy(o2[:], t2[:])
                nc.sync.dma_start(out_d[ch, 128:NT, :], o2[:])

    nc.compile()
    return nc


# ---------------------------------------------------------------- fallback
def _numpy_reference(P, c, alpha, alive, z, csg, width, height):
    """Direct numpy port of reference.py (csg-capable); slow but exact."""
    P = np.asarray(P, np.float32)
    sig = 1.0 / (1.0 + np.exp(-np.asarray(alive, np.float64)))
    eff_alpha = np.where(sig > 0.1, np.asarray(alpha, np.float64), 0.0)
    order = np.argsort(np.asarray(z, np.float64), kind='stable')
    P_s, c_s = P[order], np.asarray(c, np.float64)[order]
    a_s, csg_s = eff_alpha[order], np.asarray(csg, bool)[order]
    poly = _bezier_to_polyline(P_s.astype(np.float64))
    a = poly
    b = np.roll(poly, -1, axis=1)
    y = np.linspace(0, 1, height)
    x = np.linspace(0, 1, width)
    gx, gy = np.meshgrid(x, y)
    p = np.stack([gx, gy], -1)[None, None]              # (1,1,H,W,2)
    av = a[:, :, None, None, :]
    bv = b[:, :, None, None, :]
    ab = bv - av
    ap = p - av
    t = np.clip((ap*ab).sum(-1) / ((ab*ab).sum(-1) + EPS), 0, 1)
    diff = p - (av + t[..., None]*ab)
    dist = np.sqrt((diff*diff).sum(-1).min(1) + EPS)
    ay_, by_, py_ = av[..., 1], bv[..., 1], p[..., 1]
    ax_, bx_, px_ = av[..., 0], bv[..., 0], p[..., 0]
    up = (ay_ <= py_) & (py_ < by_)
    dn = (ay_ > py_) & (py_ >= by_)
    left = (bx_-ax_)*(py_-ay_) - (px_-ax_)*(by_-ay_) > 0
    w = np.where(up & left, 1.0, 0.0) + np.where(dn & ~left, -1.0, 0.0)
    wn = w.sum(1)
    sdf = np.where(wn != 0, -dist, dist)
    cov = 1.0/(1.0 + np.exp(sdf/0.01))
    la_all = cov * a_s[:, None, None]
    rgb = np.zeros((height, width, 3))
    ca = np.zeros((height, width, 1))
    for s in range(len(a_s)):
        la = la_all[s][..., None]
        if csg_s[s]:
            ca2 = ca*(1-la)
            rgb = rgb * (ca2 > 0)
            ca = ca2
        else:
            out_a = la + ca*(1-la)
            safe = np.where(out_a > 0, out_a, 1.0)
            rgb = np.where(out_a > 0, (c_s[s]*la + rgb*ca*(1-la))/safe, 0.0)
            ca = out_a
    return np.clip(rgb*ca, 0, 1).astype(np.float32)


# ------------------------------------------------------------------ driver
ALPHA_S = None
COL_S = None
LAST_RESULT = None


def kernel(P, c, alpha, alive, z, csg, width, height):
    global ALPHA_S, COL_S
    width = int(width)
    height = int(height)
    if width != HW or height != HW or np.asarray(csg).any():
        return _numpy_reference(P, c, alpha, alive, z, csg, width, height)

    pre = _precompute(P, c, alpha, alive, z, csg)
    ALPHA_S = [float(v) for v in pre['a_s']]
    COL_S = [[float(v) for v in row] for row in pre['c_s']]

    from concourse.bass_utils import run_bass_kernel_spmd

    nc = _build_program()

    Wfull = pre['Wfull']                      # (384, 3cb, 3k, 960)
    xfeat = pre['xfeat']                      # (3, 384)
    ck = pre['ck']                            # (384, 8)
    ident = np.eye(128, dtype=np.float32)
    xf = np.empty((3, CB, 128), np.float32)
    for cb in range(CB):
        xf[:, cb, :] = xfeat[:, cb*128:(cb+1)*128]
    cvals = ([EPS, -100.0] + [-a for a in ALPHA_S]
             + [ALPHA_S[s] * COL_S[s][ch] for s in range(N) for ch in range(3)])
    consts = np.broadcast_to(
        np.asarray(cvals, np.float32)[None, :], (128, 34)).copy()

    in_maps = []
    for core in range(NCORES):
        rows = slice(core*RPC, (core+1)*RPC)
        wcore = np.ascontiguousarray(
            Wfull[rows].transpose(0, 2, 1, 3))    # (48, 3k, 3cb, 960)
        ckcore = np.broadcast_to(
            ck[rows].reshape(1, RPC*N), (128, RPC*N)).copy()
        in_maps.append(dict(w=wcore, xfeat=xf, ck=ckcore, ident=ident,
                            consts=consts))

    trace = bool(int(os.environ.get('DIFFRAST_TRACE', '0')))
    res = run_bass_kernel_spmd(nc, in_maps, core_ids=list(range(NCORES)),
                               trace=trace)
    global LAST_RESULT
    LAST_RESULT = res

    img = np.empty((HW, HW, 3), np.float32)
    for core in range(NCORES):
        o = res.results[core]['out']          # (3, 144, 128)
        o = o.reshape(3, RPC, CB, 128).transpose(1, 2, 3, 0)  # (48,3,128,3)
        img[core*RPC:(core+1)*RPC] = o.reshape(RPC, HW, 3)
    return img


# revision 23
# speedup vs baseline: 4.9492x; 4.9492x over previous
"""Trainium2 Bass kernel for the soft Bezier rasterizer (nn_DiffRasterizer).

Contract: kernel(**inputs) takes FULL unsharded inputs (as produced by
reference.setup_inputs()) and returns the FULL (384,384,3) float32 image.

Strategy (pixel-spatial sharding, zero cross-core communication):
  * Core c owns image rows c::8 (strided; keeps per-row-index culled
    instruction shapes identical across the SPMD cores while adjacent
    rows share nearly identical cull lists).
  * Every per-(pixel,segment) quantity is a quadratic in px along a row,
    so the host bakes per (row, col-block) weight matrices over the
    feature vector [dx^2, dx, 1] (dx re-centered per 128-col block).
    The weights/features are 3-way bf16 split (6 product terms, K=18)
    so the PE evaluates them in ONE full-rate bf16 pass with fp32 PSUM
    accumulation -- matching fp32 matmul accuracy at ~4x the speed.
  * Per-row culling: distance candidates keep only segments/vertices
    within DTH of the row (sigmoid(-DTH/0.01) ~ 1e-7 -> invisible);
    winding keeps only segments whose y-interval straddles the row.
    Blocks per (row,cb): [R'(8*pe) | E(8*pe) | D2(8*pv) | C2(8*pc)]
      R' = sqrt(1e9)*(t*-0.5);  E = perp dist^2;  D2 = vertex dist^2
      C2 = 1e25 * cross  (sign-preserving, masked rows only)
    seg candidate = E + relu(R'^2 - 0.25e9); min(cand, D2) == min dist^2
    winding: ssum = sum sign(C2); inside <=> ssum != -sum(sigma)
  * Per-tile work: 1 bf16 matmul (2 if wide), ACT {Square, Relu, Copy,
    Sign} (all share one ACT table -> no table reloads), DVE {add,
    row-level min/sum reduces}. sqrt/sigmoid/compare smalls batched
    once at the end over (128,1152). Premultiplied-alpha composite.
"""
import sys
import os
import numpy as np

for _p in ('/opt/trn_rl_repo',):
    if _p not in sys.path and os.path.isdir(_p):
        sys.path.insert(0, _p)

import ml_dtypes

BF16 = ml_dtypes.bfloat16

N = 8            # shapes
S = 30           # polyline samples per shape
HW = 384         # image height == width
EPS = 1e-8
RSC = float(np.sqrt(1e9))   # R' scale
PEN = np.float32(0.25e9)    # relu threshold = 0.25*RSC^2
CSC = 1e25                  # cross scale (sign saturation)
BIGD = 1e6                  # padding distance^2 (far -> cov 0)
DTH = 0.14                  # cull distance (sigmoid(-14) = 8e-7)
NCORES = 8
RPC = HW // NCORES          # rows per core = 48
CB = 3                      # 128-wide col blocks per row
NT = RPC * CB               # pixel tiles per core = 144
NSMALL = RPC * CB * N       # 1152 end-phase elements per partition


# ---------------------------------------------------------------- host math
def _bezier_to_polyline(cp, n_samples=S):
    t_global = np.linspace(0.0, 4.0 - 4.0 / n_samples, n_samples)
    seg = np.clip(np.floor(t_global).astype(np.int64), 0, 3)
    t = t_global - seg
    ti = 1.0 - t
    basis = np.stack([ti**3, 3*ti**2*t, 3*ti*t**2, t**3], axis=-1)
    idx = np.stack([seg*3, seg*3+1, seg*3+2, (seg*3+3) % 12], axis=-1)
    gathered = cp[:, idx, :]
    return np.einsum('sk,mskd->msd', basis, gathered)


def _split3(x):
    xh = x.astype(BF16).astype(np.float64)
    xm = (x - xh).astype(BF16).astype(np.float64)
    xl = (x - xh - xm).astype(BF16).astype(np.float64)
    return xh, xm, xl


# K-stack order: terms (Xh*Wh),(Xh*Wm),(Xm*Wh),(Xh*Wl),(Xm*Wm),(Xl*Wh)
_XTERM = [0, 0, 1, 0, 1, 2]
_WTERM = [0, 1, 0, 2, 1, 0]


def _precompute(P, c, alpha, alive, z, csg):
    P = np.asarray(P, np.float64)
    sig_alive = 1.0 / (1.0 + np.exp(-np.asarray(alive, np.float64)))
    active = sig_alive > 0.1
    eff_alpha = np.where(active, np.asarray(alpha, np.float64), 0.0)
    order = np.argsort(np.asarray(z, np.float64), kind='stable')
    P_s = P[order]
    c_s = np.asarray(c, np.float64)[order]
    a_s = eff_alpha[order]

    poly = _bezier_to_polyline(P_s).astype(np.float32).astype(np.float64)
    a = poly
    b = np.roll(poly, -1, axis=1)
    ax, ay = a[..., 0].ravel(), a[..., 1].ravel()      # (240,) m-major
    bx, by = b[..., 0].ravel(), b[..., 1].ravel()
    abx, aby = bx - ax, by - ay
    ab2e = abx**2 + aby**2 + EPS
    inv = 1.0 / ab2e
    ylo = np.minimum(ay, by)
    yhi = np.maximum(ay, by)

    y = np.linspace(0.0, 1.0, HW)
    x = np.linspace(0.0, 1.0, HW)
    px0 = np.array([x[cb*128:(cb+1)*128].mean() for cb in range(CB)])
    dxf = x - np.repeat(px0, 128)
    xfeat = np.stack([dxf**2, dxf, np.ones_like(dxf)], 0)       # (3,384) f64

    # cull lists per row (global), then shared pads per row-index i
    elists, vlists, clists = [], [], []
    for r in range(HW):
        py = y[r]
        erel = (py > ylo - DTH) & (py < yhi + DTH)
        vrel = np.abs(ay - py) <= DTH
        crel = ((ay <= py) & (py < by)) | ((ay > py) & (py >= by))
        elists.append([np.nonzero(erel.reshape(N, S)[m])[0] for m in range(N)])
        vlists.append([np.nonzero(vrel.reshape(N, S)[m])[0] for m in range(N)])
        clists.append([np.nonzero(crel.reshape(N, S)[m])[0] for m in range(N)])

    pe = np.zeros(RPC, np.int64)
    pv = np.zeros(RPC, np.int64)
    pc = np.zeros(RPC, np.int64)
    for i in range(RPC):
        rows = [i*NCORES + cc for cc in range(NCORES)]
        pe[i] = max(1, max(len(elists[r][m]) for r in rows for m in range(N)))
        pv[i] = max(1, max(len(vlists[r][m]) for r in rows for m in range(N)))
        pc[i] = max(1, max(len(clists[r][m]) for r in rows for m in range(N)))
    cols = 16*pe + 8*pv + 8*pc
    maxw = int(cols.max())

    # per (row, cb): quadratic coefficient rows [A,B,C] for each column,
    # f64, re-centered per col-block; then split3 -> (18, cols) bf16.
    sigma_all = np.zeros((HW, N*S))
    for r in range(HW):
        py = y[r]
        up = (ay <= py) & (py < by)
        dn = (ay > py) & (py >= by)
        sigma_all[r] = np.where(up, 1.0, 0.0) - np.where(dn, 1.0, 0.0)
    ck = (-sigma_all.reshape(HW, N, S).sum(-1)).astype(np.float32)  # (384,8)

    def row_coeffs(r, i):
        """(3, cols_i) f64 coefficient matrix for global row r, index i."""
        py = y[r]
        e = aby*py - abx*ax - aby*ay                    # D1 = abx*px + e
        npe, npv, npc = pe[i], pv[i], pc[i]
        C = np.zeros((3, cols[i]))
        for m in range(N):
            el, vl, cl = elists[r][m], vlists[r][m], clists[r][m]
            base_r = m*npe
            base_e = 8*npe + m*npe
            base_v = 16*npe + m*npv
            base_c = 16*npe + 8*npv + m*npc
            sel = m*S + el
            # R' = (D1*inv - 0.5)*RSC
            C[0, base_r:base_r+len(el)] = 0.0
            C[1, base_r:base_r+len(el)] = (abx[sel]*inv[sel])*RSC
            C[2, base_r:base_r+len(el)] = (e[sel]*inv[sel] - 0.5)*RSC
            # E = D2 - D1^2*inv
            C[0, base_e:base_e+len(el)] = 1.0 - abx[sel]**2*inv[sel]
            C[1, base_e:base_e+len(el)] = -2*ax[sel] - 2*abx[sel]*e[sel]*inv[sel]
            C[2, base_e:base_e+len(el)] = (ax[sel]**2 + (py - ay[sel])**2
                                           - e[sel]**2*inv[sel])
            C[2, base_e+len(el):base_e+npe] = BIGD      # pads
            if len(el) < npe:
                C[2, base_r+len(el):base_r+npe] = 0.0
            # D2 = px^2 - 2 ax px + ax^2 + (py-ay)^2
            sv = m*S + vl
            C[0, base_v:base_v+len(vl)] = 1.0
            C[1, base_v:base_v+len(vl)] = -2*ax[sv]
            C[2, base_v:base_v+len(vl)] = ax[sv]**2 + (py - ay[sv])**2
            C[2, base_v+len(vl):base_v+npv] = BIGD
            # C2 = CSC * cross ;  cross = -aby*px + (abx*(py-ay) + ax*aby)
            sc = m*S + cl
            C[0, base_c:base_c+len(cl)] = 0.0
            C[1, base_c:base_c+len(cl)] = -aby[sc]*CSC
            C[2, base_c:base_c+len(cl)] = (abx[sc]*(py - ay[sc])
                                           + ax[sc]*aby[sc])*CSC
        return C

    # Re-center per col-block and split
    Wcore = np.zeros((NCORES, RPC, CB, 18, maxw), BF16)
    for i in range(RPC):
        for cc in range(NCORES):
            r = i*NCORES + cc
            Cq = row_coeffs(r, i)                       # (3, cols_i)
            A, B_, C0 = Cq[0], Cq[1], Cq[2]
            for cb in range(CB):
                p0 = px0[cb]
                W = np.stack([A, 2*A*p0 + B_, A*p0*p0 + B_*p0 + C0], 0)
                Wh, Wm, Wl = _split3(W)
                Wparts = (Wh, Wm, Wl)
                for t6 in range(6):
                    Wcore[cc, i, cb, t6*3:(t6+1)*3, :cols[i]] = \
                        Wparts[_WTERM[t6]].astype(BF16)

    Xh, Xm, Xl = _split3(xfeat)
    Xparts = (Xh, Xm, Xl)
    X18 = np.zeros((18, CB, 128), BF16)
    for cb in range(CB):
        for t6 in range(6):
            X18[t6*3:(t6+1)*3, cb, :] = \
                Xparts[_XTERM[t6]][:, cb*128:(cb+1)*128].astype(BF16)

    # ckall per core: (128, i, cb, s) replicated partitions
    ckall = np.zeros((NCORES, 128, RPC, CB, N), np.float32)
    for cc in range(NCORES):
        for i in range(RPC):
            ckall[cc, :, i, :, :] = ck[i*NCORES + cc][None, None, :]

    Wcore = np.ascontiguousarray(Wcore.transpose(0, 1, 3, 2, 4))
    return dict(Wcore=Wcore,   # (NCORES, RPC, 18, CB, maxw)
                X18=X18, ckall=ckall.reshape(NCORES, 128, NSMALL),
                pe=pe, pv=pv, pc=pc, cols=cols, maxw=maxw,
                c_s=c_s.astype(np.float32), a_s=a_s.astype(np.float32),
                poly=poly.astype(np.float32))


# ------------------------------------------------------------- bass program
def _build_program(pe, pv, pc, cols, maxw):
    import concourse.bass as bass
    import concourse.bacc as bacc
    import concourse.mybir as mybir
    from concourse import tile

    dt = mybir.dt.float32
    bt = mybir.dt.bfloat16
    AF = mybir.ActivationFunctionType
    ALU = mybir.AluOpType
    AX = mybir.AxisListType

    nc = bacc.Bacc()
    w_d = nc.declare_dram_parameter("w", [RPC, 18, CB, maxw], bt,
                                    isOutput=False)
    xf_d = nc.declare_dram_parameter("xfeat", [18, CB, 128], bt,
                                     isOutput=False)
    ck_d = nc.declare_dram_parameter("ck", [128, NSMALL], dt, isOutput=False)
    ident_d = nc.declare_dram_parameter("ident", [128, 128], dt,
                                        isOutput=False)
    cst_d = nc.declare_dram_parameter("consts", [128, 8], dt, isOutput=False)
    out_d = nc.declare_dram_parameter("out", [3, NT, 128], dt, isOutput=True)

    with tile.TileContext(nc) as tc:
        with (
            tc.tile_pool(name="const", bufs=1) as cpool,
            tc.tile_pool(name="wpool", bufs=3) as wpool,
            tc.tile_pool(name="work", bufs=4) as work,
            tc.tile_pool(name="slabs", bufs=1) as slabs,
            tc.tile_pool(name="psA", bufs=4, space=bass.MemorySpace.PSUM) as psa,
            tc.tile_pool(name="psB", bufs=2, space=bass.MemorySpace.PSUM) as psb,
            tc.tile_pool(name="psT", bufs=1, space=bass.MemorySpace.PSUM) as pst,
        ):
            xfeat = cpool.tile([18, CB, 128], bt)
            nc.sync.dma_start(xfeat[:], xf_d[:])
            ckt = cpool.tile([128, NSMALL], dt)
            nc.sync.dma_start(ckt[:], ck_d[:])
            ident = cpool.tile([128, 128], dt)
            nc.sync.dma_start(ident[:], ident_d[:])
            cst = cpool.tile([128, 8], dt)
            nc.sync.dma_start(cst[:], cst_d[:])
            c_eps = cst[:, 0:1]

            la_all = slabs.tile([128, N, NT], dt)
            mindall = slabs.tile([128, NSMALL], dt)
            ssumall = slabs.tile([128, NSMALL], dt)

            for i in range(RPC):
                npe, npv, npc = int(pe[i]), int(pv[i]), int(pc[i])
                ci = int(cols[i])
                wt = wpool.tile([18, CB, maxw], bt, tag="w")
                nc.sync.dma_start(wt[:], w_d[i])
                slab = work.tile([128, CB, N, npe + npv], dt, tag="slab")
                s1 = work.tile([128, CB, N, npc], dt, tag="s1")
                for cb in range(CB):
                    if ci <= 512:
                        pA = psa.tile([128, ci], dt, tag="pA")
                        nc.tensor.matmul(pA[:], xfeat[:, cb, :], wt[:, cb, 0:ci],
                                         start=True, stop=True)
                        ap_R = pA[:, 0:8*npe]
                        ap_E = pA[:, 8*npe:16*npe]
                        ap_D = pA[:, 16*npe:16*npe+8*npv]
                        ap_C = pA[:, 16*npe+8*npv:ci]
                    else:
                        pA = psa.tile([128, 16*npe], dt, tag="pA")
                        pB = psb.tile([128, ci - 16*npe], dt, tag="pB")
                        nc.tensor.matmul(pA[:], xfeat[:, cb, :],
                                         wt[:, cb, 0:16*npe],
                                         start=True, stop=True)
                        nc.tensor.matmul(pB[:], xfeat[:, cb, :],
                                         wt[:, cb, 16*npe:ci],
                                         start=True, stop=True)
                        ap_R = pA[:, 0:8*npe]
                        ap_E = pA[:, 8*npe:16*npe]
                        ap_D = pB[:, 0:8*npv]
                        ap_C = pB[:, 8*npv:8*npv+8*npc]

                    r2 = work.tile([128, 8*npe], dt, tag="r2")
                    nc.scalar.activation(r2[:], ap_R, AF.Square)
                    pen = work.tile([128, 8*npe], dt, tag="pen")
                    nc.scalar.activation(pen[:], r2[:], AF.Relu,
                                         bias=cst[:, 1:2])
                    # seg candidates -> slab[..., 0:npe]
                    nc.vector.tensor_tensor(
                        slab[:, cb, :, 0:npe], ap_E, pen[:], ALU.add)
                    # vertex candidates -> slab[..., npe:npe+npv]
                    nc.scalar.activation(slab[:, cb, :, npe:npe+npv],
                                         ap_D, AF.Copy)
                    nc.scalar.activation(s1[:, cb], ap_C, AF.Sign)

                # row-level reduces straight into the end-phase slabs
                nc.vector.tensor_reduce(
                    mindall[:, i*CB*N:(i+1)*CB*N], slab[:], AX.X, ALU.min)
                nc.vector.tensor_reduce(
                    ssumall[:, i*CB*N:(i+1)*CB*N], s1[:], AX.X, ALU.add)

            # ---- end phase, batched over (128, 1152)
            m0 = slabs.tile([128, NSMALL], dt)
            nc.vector.tensor_scalar_max(m0[:], mindall[:], 0.0)
            sd = slabs.tile([128, NSMALL], dt)
            nc.scalar.activation(sd[:], m0[:], AF.Sqrt, bias=c_eps)
            ins = slabs.tile([128, NSMALL], dt)
            nc.vector.tensor_tensor(ins[:], ssumall[:], ckt[:], ALU.not_equal)
            sgn = slabs.tile([128, NSMALL], dt)
            nc.vector.tensor_scalar(sgn[:], ins[:], -2.0, 1.0,
                                    ALU.mult, ALU.add)
            sdf = slabs.tile([128, NSMALL], dt)
            nc.vector.tensor_tensor(sdf[:], sgn[:], sd[:], ALU.mult)
            # cov = sigmoid(-100*sdf); source order (i,cb,s) -> la_all[s, t]
            la_t = la_all[:].rearrange("p n (i cb) -> p i cb n", cb=CB)
            nc.scalar.activation(la_t, sdf[:], AF.Sigmoid, scale=-100.0)

            # ---- composite: prgb' = prgb + (alpha_s*cov)*(col_ch - prgb)
            prgb = slabs.tile([128, 3, NT], dt)
            nc.vector.memset(prgb[:], 0.0)
            for s in range(N):
                la_s = la_all[:, s, :]
                for ch in range(3):
                    diff = work.tile([128, NT], dt, tag="diff")
                    nc.scalar.activation(diff[:], prgb[:, ch, :], AF.Copy,
                                         scale=float(-ALPHA_S[s]),
                                         bias=float(ALPHA_S[s] * COL_S[s][ch]))
                    m = work.tile([128, NT], dt, tag="m")
                    nc.vector.tensor_tensor(m[:], la_s, diff[:], ALU.mult)
                    nc.vector.tensor_tensor(prgb[:, ch, :], prgb[:, ch, :],
                                            m[:], ALU.add)
            for ch in range(3):
                nc.vector.tensor_scalar(prgb[:, ch, :], prgb[:, ch, :],
                                        0.0, 1.0, ALU.max, ALU.min)

            # ---- output transpose: (128 p, 144 t) -> (144 t, 128 p) per ch
            for ch in range(3):
                t1 = pst.tile([128, 128], dt, tag="t1")
                nc.tensor.transpose(t1[:], prgb[:, ch, 0:128], ident[:])
                o1 = work.tile([128, 128], dt, tag="o1")
                nc.vector.tensor_copy(o1[:], t1[:])
                nc.sync.dma_start(out_d[ch, 0:128, :], o1[:])
                t2 = pst.tile([16, 128], dt, tag="t2")
                nc.tensor.transpose(t2[:], prgb[:, ch, 128:NT], ident[:])
                o2 = work.tile([16, 128], dt, tag="o2")
                nc.vector.tensor_copy(o2[:], t2[:])
                nc.sync.dma_start(out_d[ch, 128:NT, :], o2[:])

    nc.compile()
    return nc


# ---------------------------------------------------------------- fallback
def _numpy_reference(P, c, alpha, alive, z, csg, width, height):
    """Direct numpy port of reference.py (csg-capable); slow but exact."""
    P = np.asarray(P, np.float32)
    sig = 1.0 / (1.0 + np.exp(-np.asarray(alive, np.float64)))
    eff_alpha = np.where(sig > 0.1, np.asarray(alpha, np.float64), 0.0)
    order = np.argsort(np.asarray(z, np.float64), kind='stable')
    P_s, c_s = P[order], np.asarray(c, np.float64)[order]
    a_s, csg_s = eff_alpha[order], np.asarray(csg, bool)[order]
    poly = _bezier_to_polyline(P_s.astype(np.float64))
    a = poly
    b = np.roll(poly, -1, axis=1)
    y = np.linspace(0, 1, height)
    x = np.linspace(0, 1, width)
    gx, gy = np.meshgrid(x, y)
    p = np.stack([gx, gy], -1)[None, None]
    av = a[:, :, None, None, :]
    bv = b[:, :, None, None, :]
    ab = bv - av
    ap = p - av
    t = np.clip((ap*ab).sum(-1) / ((ab*ab).sum(-1) + EPS), 0, 1)
    diff = p - (av + t[..., None]*ab)
    dist = np.sqrt((diff*diff).sum(-1).min(1) + EPS)
    ay_, by_, py_ = av[..., 1], bv[..., 1], p[..., 1]
    ax_, bx_, px_ = av[..., 0], bv[..., 0], p[..., 0]
    up = (ay_ <= py_) & (py_ < by_)
    dn = (ay_ > py_) & (py_ >= by_)
    left = (bx_-ax_)*(py_-ay_) - (px_-ax_)*(by_-ay_) > 0
    w = np.where(up & left, 1.0, 0.0) + np.where(dn & ~left, -1.0, 0.0)
    wn = w.sum(1)
    sdf = np.where(wn != 0, -dist, dist)
    cov = 1.0/(1.0 + np.exp(sdf/0.01))
    la_all = cov * a_s[:, None, None]
    rgb = np.zeros((height, width, 3))
    ca = np.zeros((height, width, 1))
    for s in range(len(a_s)):
        la = la_all[s][..., None]
        if csg_s[s]:
            ca2 = ca*(1-la)
            rgb = rgb * (ca2 > 0)
            ca = ca2
        else:
            out_a = la + ca*(1-la)
            safe = np.where(out_a > 0, out_a, 1.0)
            rgb = np.where(out_a > 0, (c_s[s]*la + rgb*ca*(1-la))/safe, 0.0)
            ca = out_a
    return np.clip(rgb*ca, 0, 1).astype(np.float32)


# ------------------------------------------------------------------ driver
ALPHA_S = None
COL_S = None
LAST_RESULT = None


def kernel(P, c, alpha, alive, z, csg, width, height):
    global ALPHA_S, COL_S, LAST_RESULT
    width = int(width)
    height = int(height)
    if width != HW or height != HW or np.asarray(csg).any():
        return _numpy_reference(P, c, alpha, alive, z, csg, width, height)

    pre = _precompute(P, c, alpha, alive, z, csg)
    ALPHA_S = [float(v) for v in pre['a_s']]
    COL_S = [[float(v) for v in row] for row in pre['c_s']]

    from concourse.bass_utils import run_bass_kernel_spmd

    nc = _build_program(pre['pe'], pre['pv'], pre['pc'], pre['cols'],
                        pre['maxw'])

    ident = np.eye(128, dtype=np.float32)
    cvals = [EPS, -float(PEN)] + [0.0]*6
    consts = np.broadcast_to(
        np.asarray(cvals, np.float32)[None, :], (128, 8)).copy()

    in_maps = []
    for cc in range(NCORES):
        in_maps.append(dict(w=np.ascontiguousarray(pre['Wcore'][cc]),
                            xfeat=pre['X18'], ck=pre['ckall'][cc],
                            ident=ident, consts=consts))

    trace = bool(int(os.environ.get('DIFFRAST_TRACE', '0')))
    res = run_bass_kernel_spmd(nc, in_maps, core_ids=list(range(NCORES)),
                               trace=trace)
    LAST_RESULT = res

    img = np.empty((HW, HW, 3), np.float32)
    for cc in range(NCORES):
        o = res.results[cc]['out']            # (3, 144, 128)
        o = o.reshape(3, RPC, CB, 128).transpose(1, 2, 3, 0)  # (48,3,128,3)
        img[cc::NCORES] = o.reshape(RPC, HW, 3)
    return img


# revision 25
# speedup vs baseline: 5.3784x; 1.0867x over previous
"""Trainium2 Bass kernel for the soft Bezier rasterizer (nn_DiffRasterizer).

Contract: kernel(**inputs) takes FULL unsharded inputs (as produced by
reference.setup_inputs()) and returns the FULL (384,384,3) float32 image.

Strategy (pixel-spatial sharding, zero cross-core communication):
  * Core c owns image rows c::8 (strided; keeps per-row-index culled
    instruction shapes identical across the SPMD cores while adjacent
    rows share nearly identical cull lists).
  * Every per-(pixel,segment) quantity is a quadratic in px along a row,
    so the host bakes per (row, col-block) weight matrices over the
    feature vector [dx^2, dx, 1] (dx re-centered per 128-col block).
    The weights/features are 3-way bf16 split (6 product terms, K=18)
    so the PE evaluates them in ONE full-rate bf16 pass with fp32 PSUM
    accumulation -- matching fp32 matmul accuracy at ~4x the speed.
  * Per-row culling: distance candidates keep only segments/vertices
    within DTH of the row (sigmoid(-DTH/0.01) ~ 1e-7 -> invisible);
    winding keeps only segments whose y-interval straddles the row.
    Blocks per (row,cb): [R'(8*pe) | E(8*pe) | D2(8*pv) | C2(8*pc)]
      R' = sqrt(1e9)*(t*-0.5);  E = perp dist^2;  D2 = vertex dist^2
      C2 = 1e25 * cross  (sign-preserving, masked rows only)
    seg candidate = E + relu(R'^2 - 0.25e9); min(cand, D2) == min dist^2
    winding: ssum = sum sign(C2); inside <=> ssum != -sum(sigma)
  * Per-tile work: 1 bf16 matmul (2 if wide), ACT {Square, Relu, Copy,
    Sign} (all share one ACT table -> no table reloads), DVE {add,
    row-level min/sum reduces}. sqrt/sigmoid/compare smalls batched
    once at the end over (128,1152). Premultiplied-alpha composite.
"""
import sys
import os
import numpy as np

for _p in ('/opt/trn_rl_repo',):
    if _p not in sys.path and os.path.isdir(_p):
        sys.path.insert(0, _p)

import ml_dtypes

BF16 = ml_dtypes.bfloat16

N = 8            # shapes
S = 30           # polyline samples per shape
HW = 384         # image height == width
EPS = 1e-8
RSC = float(np.sqrt(1e9))   # R' scale
PEN = np.float32(0.25e9)    # relu threshold = 0.25*RSC^2
CSC = 1e25                  # cross scale (sign saturation)
BIGD = 1e6                  # padding distance^2 (far -> cov 0)
DTH = 0.14                  # cull distance (sigmoid(-14) = 8e-7)
NCORES = 8
RPC = HW // NCORES          # rows per core = 48
CB = 3                      # 128-wide col blocks per row
NT = RPC * CB               # pixel tiles per core = 144
NSMALL = RPC * CB * N       # 1152 end-phase elements per partition


# ---------------------------------------------------------------- host math
def _bezier_to_polyline(cp, n_samples=S):
    t_global = np.linspace(0.0, 4.0 - 4.0 / n_samples, n_samples)
    seg = np.clip(np.floor(t_global).astype(np.int64), 0, 3)
    t = t_global - seg
    ti = 1.0 - t
    basis = np.stack([ti**3, 3*ti**2*t, 3*ti*t**2, t**3], axis=-1)
    idx = np.stack([seg*3, seg*3+1, seg*3+2, (seg*3+3) % 12], axis=-1)
    gathered = cp[:, idx, :]
    return np.einsum('sk,mskd->msd', basis, gathered)


def _split3(x):
    xh = x.astype(BF16).astype(np.float64)
    xm = (x - xh).astype(BF16).astype(np.float64)
    xl = (x - xh - xm).astype(BF16).astype(np.float64)
    return xh, xm, xl


# K-stack order: terms (Xh*Wh),(Xh*Wm),(Xm*Wh),(Xh*Wl),(Xm*Wm),(Xl*Wh)
_XTERM = [0, 0, 1, 0, 1, 2]
_WTERM = [0, 1, 0, 2, 1, 0]


def _precompute(P, c, alpha, alive, z, csg):
    P = np.asarray(P, np.float64)
    sig_alive = 1.0 / (1.0 + np.exp(-np.asarray(alive, np.float64)))
    active = sig_alive > 0.1
    eff_alpha = np.where(active, np.asarray(alpha, np.float64), 0.0)
    order = np.argsort(np.asarray(z, np.float64), kind='stable')
    P_s = P[order]
    c_s = np.asarray(c, np.float64)[order]
    a_s = eff_alpha[order]

    poly = _bezier_to_polyline(P_s).astype(np.float32).astype(np.float64)
    a = poly
    b = np.roll(poly, -1, axis=1)
    ax, ay = a[..., 0].ravel(), a[..., 1].ravel()      # (240,) m-major
    bx, by = b[..., 0].ravel(), b[..., 1].ravel()
    abx, aby = bx - ax, by - ay
    ab2e = abx**2 + aby**2 + EPS
    inv = 1.0 / ab2e
    ylo = np.minimum(ay, by)
    yhi = np.maximum(ay, by)

    y = np.linspace(0.0, 1.0, HW)
    x = np.linspace(0.0, 1.0, HW)
    px0 = np.array([x[cb*128:(cb+1)*128].mean() for cb in range(CB)])
    dxf = x - np.repeat(px0, 128)
    xfeat = np.stack([dxf**2, dxf, np.ones_like(dxf)], 0)       # (3,384) f64

    # cull lists per row (global), then shared pads per row-index i
    elists, vlists, clists = [], [], []
    for r in range(HW):
        py = y[r]
        erel = (py > ylo - DTH) & (py < yhi + DTH)
        vrel = np.abs(ay - py) <= DTH
        crel = ((ay <= py) & (py < by)) | ((ay > py) & (py >= by))
        elists.append([np.nonzero(erel.reshape(N, S)[m])[0] for m in range(N)])
        vlists.append([np.nonzero(vrel.reshape(N, S)[m])[0] for m in range(N)])
        clists.append([np.nonzero(crel.reshape(N, S)[m])[0] for m in range(N)])

    pe = np.zeros(RPC, np.int64)
    pv = np.zeros(RPC, np.int64)
    pc = np.zeros(RPC, np.int64)
    for i in range(RPC):
        rows = [i*NCORES + cc for cc in range(NCORES)]
        pe[i] = max(1, max(len(elists[r][m]) for r in rows for m in range(N)))
        pv[i] = max(1, max(len(vlists[r][m]) for r in rows for m in range(N)))
        pc[i] = max(1, max(len(clists[r][m]) for r in rows for m in range(N)))
    cols = 16*pe + 8*pv + 8*pc
    maxw = int(cols.max())

    # per (row, cb): quadratic coefficient rows [A,B,C] for each column,
    # f64, re-centered per col-block; then split3 -> (18, cols) bf16.
    sigma_all = np.zeros((HW, N*S))
    for r in range(HW):
        py = y[r]
        up = (ay <= py) & (py < by)
        dn = (ay > py) & (py >= by)
        sigma_all[r] = np.where(up, 1.0, 0.0) - np.where(dn, 1.0, 0.0)
    ck = (-sigma_all.reshape(HW, N, S).sum(-1)).astype(np.float32)  # (384,8)

    def row_coeffs(r, i):
        """(3, cols_i) f64 coefficient matrix for global row r, index i."""
        py = y[r]
        e = aby*py - abx*ax - aby*ay                    # D1 = abx*px + e
        npe, npv, npc = pe[i], pv[i], pc[i]
        C = np.zeros((3, cols[i]))
        for m in range(N):
            el, vl, cl = elists[r][m], vlists[r][m], clists[r][m]
            base_r = m*npe
            base_e = 8*npe + m*npe
            base_v = 16*npe + m*npv
            base_c = 16*npe + 8*npv + m*npc
            sel = m*S + el
            # R' = (D1*inv - 0.5)*RSC
            C[0, base_r:base_r+len(el)] = 0.0
            C[1, base_r:base_r+len(el)] = (abx[sel]*inv[sel])*RSC
            C[2, base_r:base_r+len(el)] = (e[sel]*inv[sel] - 0.5)*RSC
            # E = D2 - D1^2*inv
            C[0, base_e:base_e+len(el)] = 1.0 - abx[sel]**2*inv[sel]
            C[1, base_e:base_e+len(el)] = -2*ax[sel] - 2*abx[sel]*e[sel]*inv[sel]
            C[2, base_e:base_e+len(el)] = (ax[sel]**2 + (py - ay[sel])**2
                                           - e[sel]**2*inv[sel])
            C[2, base_e+len(el):base_e+npe] = BIGD      # pads
            if len(el) < npe:
                C[2, base_r+len(el):base_r+npe] = 0.0
            # D2 = px^2 - 2 ax px + ax^2 + (py-ay)^2
            sv = m*S + vl
            C[0, base_v:base_v+len(vl)] = 1.0
            C[1, base_v:base_v+len(vl)] = -2*ax[sv]
            C[2, base_v:base_v+len(vl)] = ax[sv]**2 + (py - ay[sv])**2
            C[2, base_v+len(vl):base_v+npv] = BIGD
            # C2 = CSC * cross ;  cross = -aby*px + (abx*(py-ay) + ax*aby)
            sc = m*S + cl
            C[0, base_c:base_c+len(cl)] = 0.0
            C[1, base_c:base_c+len(cl)] = -aby[sc]*CSC
            C[2, base_c:base_c+len(cl)] = (abx[sc]*(py - ay[sc])
                                           + ax[sc]*aby[sc])*CSC
        return C

    # Re-center per col-block and split
    Wcore = np.zeros((NCORES, RPC, CB, 18, maxw), BF16)
    for i in range(RPC):
        for cc in range(NCORES):
            r = i*NCORES + cc
            Cq = row_coeffs(r, i)                       # (3, cols_i)
            A, B_, C0 = Cq[0], Cq[1], Cq[2]
            for cb in range(CB):
                p0 = px0[cb]
                W = np.stack([A, 2*A*p0 + B_, A*p0*p0 + B_*p0 + C0], 0)
                Wh, Wm, Wl = _split3(W)
                Wparts = (Wh, Wm, Wl)
                for t6 in range(6):
                    Wcore[cc, i, cb, t6*3:(t6+1)*3, :cols[i]] = \
                        Wparts[_WTERM[t6]].astype(BF16)

    Xh, Xm, Xl = _split3(xfeat)
    Xparts = (Xh, Xm, Xl)
    X18 = np.zeros((18, CB, 128), BF16)
    for cb in range(CB):
        for t6 in range(6):
            X18[t6*3:(t6+1)*3, cb, :] = \
                Xparts[_XTERM[t6]][:, cb*128:(cb+1)*128].astype(BF16)

    # ckall per core: (128, i, cb, s) replicated partitions
    ckall = np.zeros((NCORES, 128, RPC, CB, N), np.float32)
    for cc in range(NCORES):
        for i in range(RPC):
            ckall[cc, :, i, :, :] = ck[i*NCORES + cc][None, None, :]

    Wcore = np.ascontiguousarray(Wcore.transpose(0, 1, 3, 2, 4))
    return dict(Wcore=Wcore,   # (NCORES, RPC, 18, CB, maxw)
                X18=X18, ckall=ckall.reshape(NCORES, 128, NSMALL),
                pe=pe, pv=pv, pc=pc, cols=cols, maxw=maxw,
                c_s=c_s.astype(np.float32), a_s=a_s.astype(np.float32),
                poly=poly.astype(np.float32))


# ------------------------------------------------------------- bass program
def _build_program(pe, pv, pc, cols, maxw):
    import concourse.bass as bass
    import concourse.bacc as bacc
    import concourse.mybir as mybir
    from concourse import tile

    dt = mybir.dt.float32
    bt = mybir.dt.bfloat16
    AF = mybir.ActivationFunctionType
    ALU = mybir.AluOpType
    AX = mybir.AxisListType

    nc = bacc.Bacc()
    w_d = nc.declare_dram_parameter("w", [RPC, 18, CB, maxw], bt,
                                    isOutput=False)
    xf_d = nc.declare_dram_parameter("xfeat", [18, CB, 128], bt,
                                     isOutput=False)
    ck_d = nc.declare_dram_parameter("ck", [128, NSMALL], dt, isOutput=False)
    ident_d = nc.declare_dram_parameter("ident", [128, 128], dt,
                                        isOutput=False)
    cst_d = nc.declare_dram_parameter("consts", [128, 8], dt, isOutput=False)
    out_d = nc.declare_dram_parameter("out", [3, NT, 128], dt, isOutput=True)

    with tile.TileContext(nc) as tc:
        with (
            tc.tile_pool(name="const", bufs=1) as cpool,
            tc.tile_pool(name="wpool", bufs=3) as wpool,
            tc.tile_pool(name="work", bufs=4) as work,
            tc.tile_pool(name="slabs", bufs=1) as slabs,
            tc.tile_pool(name="psA", bufs=4, space=bass.MemorySpace.PSUM) as psa,
            tc.tile_pool(name="psB", bufs=2, space=bass.MemorySpace.PSUM) as psb,
            tc.tile_pool(name="psT", bufs=1, space=bass.MemorySpace.PSUM) as pst,
        ):
            xfeat = cpool.tile([18, CB, 128], bt)
            nc.sync.dma_start(xfeat[:], xf_d[:])
            ckt = cpool.tile([128, NSMALL], dt)
            nc.sync.dma_start(ckt[:], ck_d[:])
            ident = cpool.tile([128, 128], dt)
            nc.sync.dma_start(ident[:], ident_d[:])
            cst = cpool.tile([128, 8], dt)
            nc.sync.dma_start(cst[:], cst_d[:])
            c_eps = cst[:, 0:1]

            la_all = slabs.tile([128, N, NT], dt)
            mindall = slabs.tile([128, NSMALL], dt)
            ssumall = slabs.tile([128, NSMALL], dt)

            for i in range(RPC):
                npe, npv, npc = int(pe[i]), int(pv[i]), int(pc[i])
                ci = int(cols[i])
                wt = wpool.tile([18, CB, maxw], bt, tag="w")
                nc.sync.dma_start(wt[:], w_d[i])
                slab = work.tile([128, CB, N, npe + npv], dt, tag="slab")
                s1 = work.tile([128, CB, N, npc], dt, tag="s1")
                for cb in range(CB):
                    if ci <= 512:
                        pA = psa.tile([128, ci], dt, tag="pA")
                        nc.tensor.matmul(pA[:], xfeat[:, cb, :], wt[:, cb, 0:ci],
                                         start=True, stop=True)
                        ap_R = pA[:, 0:8*npe]
                        ap_E = pA[:, 8*npe:16*npe]
                        ap_D = pA[:, 16*npe:16*npe+8*npv]
                        ap_C = pA[:, 16*npe+8*npv:ci]
                    else:
                        pA = psa.tile([128, 16*npe], dt, tag="pA")
                        pB = psb.tile([128, ci - 16*npe], dt, tag="pB")
                        nc.tensor.matmul(pA[:], xfeat[:, cb, :],
                                         wt[:, cb, 0:16*npe],
                                         start=True, stop=True)
                        nc.tensor.matmul(pB[:], xfeat[:, cb, :],
                                         wt[:, cb, 16*npe:ci],
                                         start=True, stop=True)
                        ap_R = pA[:, 0:8*npe]
                        ap_E = pA[:, 8*npe:16*npe]
                        ap_D = pB[:, 0:8*npv]
                        ap_C = pB[:, 8*npv:8*npv+8*npc]

                    r2 = work.tile([128, 8*npe], dt, tag="r2")
                    nc.scalar.activation(r2[:], ap_R, AF.Square)
                    pen = work.tile([128, 8*npe], dt, tag="pen")
                    nc.scalar.activation(pen[:], r2[:], AF.Relu,
                                         bias=cst[:, 1:2])
                    # seg candidates -> slab[..., 0:npe]
                    nc.vector.tensor_tensor(
                        slab[:, cb, :, 0:npe], ap_E, pen[:], ALU.add)
                    # vertex candidates -> slab[..., npe:npe+npv]
                    nc.scalar.activation(slab[:, cb, :, npe:npe+npv],
                                         ap_D, AF.Copy)
                    # sign(C2): C2 pre-scaled by 1e25 so clamp saturates
                    # to exactly +/-1 (0 stays 0)
                    nc.vector.tensor_scalar(s1[:, cb], ap_C, -1.0, 1.0,
                                            ALU.max, ALU.min)

                # row-level reduces straight into the end-phase slabs
                nc.vector.tensor_reduce(
                    mindall[:, i*CB*N:(i+1)*CB*N], slab[:], AX.X, ALU.min)
                nc.vector.tensor_reduce(
                    ssumall[:, i*CB*N:(i+1)*CB*N], s1[:], AX.X, ALU.add)

            # ---- end phase, batched over (128, 1152)
            m0 = slabs.tile([128, NSMALL], dt)
            nc.vector.tensor_scalar_max(m0[:], mindall[:], 0.0)
            sd = slabs.tile([128, NSMALL], dt)
            nc.scalar.activation(sd[:], m0[:], AF.Sqrt, bias=c_eps)
            ins = slabs.tile([128, NSMALL], dt)
            nc.vector.tensor_tensor(ins[:], ssumall[:], ckt[:], ALU.not_equal)
            sgn = slabs.tile([128, NSMALL], dt)
            nc.vector.tensor_scalar(sgn[:], ins[:], -2.0, 1.0,
                                    ALU.mult, ALU.add)
            sdf = slabs.tile([128, NSMALL], dt)
            nc.vector.tensor_tensor(sdf[:], sgn[:], sd[:], ALU.mult)
            # cov = sigmoid(-100*sdf); source order (i,cb,s) -> la_all[s, t]
            la_t = la_all[:].rearrange("p n (i cb) -> p i cb n", cb=CB)
            nc.scalar.activation(la_t, sdf[:], AF.Sigmoid, scale=-100.0)

            # ---- composite: prgb' = prgb + (alpha_s*cov)*(col_ch - prgb)
            prgb = slabs.tile([128, 3, NT], dt)
            nc.vector.memset(prgb[:], 0.0)
            for s in range(N):
                la_s = la_all[:, s, :]
                for ch in range(3):
                    diff = work.tile([128, NT], dt, tag="diff")
                    nc.vector.tensor_scalar(diff[:], prgb[:, ch, :],
                                            float(-ALPHA_S[s]),
                                            float(ALPHA_S[s] * COL_S[s][ch]),
                                            ALU.mult, ALU.add)
                    m = work.tile([128, NT], dt, tag="m")
                    nc.vector.tensor_tensor(m[:], la_s, diff[:], ALU.mult)
                    nc.vector.tensor_tensor(prgb[:, ch, :], prgb[:, ch, :],
                                            m[:], ALU.add)
            for ch in range(3):
                nc.vector.tensor_scalar(prgb[:, ch, :], prgb[:, ch, :],
                                        0.0, 1.0, ALU.max, ALU.min)

            # ---- output transpose: (128 p, 144 t) -> (144 t, 128 p) per ch
            for ch in range(3):
                t1 = pst.tile([128, 128], dt, tag="t1")
                nc.tensor.transpose(t1[:], prgb[:, ch, 0:128], ident[:])
                o1 = work.tile([128, 128], dt, tag="o1")
                nc.vector.tensor_copy(o1[:], t1[:])
                nc.sync.dma_start(out_d[ch, 0:128, :], o1[:])
                t2 = pst.tile([16, 128], dt, tag="t2")
                nc.tensor.transpose(t2[:], prgb[:, ch, 128:NT], ident[:])
                o2 = work.tile([16, 128], dt, tag="o2")
                nc.vector.tensor_copy(o2[:], t2[:])
                nc.sync.dma_start(out_d[ch, 128:NT, :], o2[:])

    nc.compile()
    return nc


# ---------------------------------------------------------------- fallback
def _numpy_reference(P, c, alpha, alive, z, csg, width, height):
    """Direct numpy port of reference.py (csg-capable); slow but exact."""
    P = np.asarray(P, np.float32)
    sig = 1.0 / (1.0 + np.exp(-np.asarray(alive, np.float64)))
    eff_alpha = np.where(sig > 0.1, np.asarray(alpha, np.float64), 0.0)
    order = np.argsort(np.asarray(z, np.float64), kind='stable')
    P_s, c_s = P[order], np.asarray(c, np.float64)[order]
    a_s, csg_s = eff_alpha[order], np.asarray(csg, bool)[order]
    poly = _bezier_to_polyline(P_s.astype(np.float64))
    a = poly
    b = np.roll(poly, -1, axis=1)
    y = np.linspace(0, 1, height)
    x = np.linspace(0, 1, width)
    gx, gy = np.meshgrid(x, y)
    p = np.stack([gx, gy], -1)[None, None]
    av = a[:, :, None, None, :]
    bv = b[:, :, None, None, :]
    ab = bv - av
    ap = p - av
    t = np.clip((ap*ab).sum(-1) / ((ab*ab).sum(-1) + EPS), 0, 1)
    diff = p - (av + t[..., None]*ab)
    dist = np.sqrt((diff*diff).sum(-1).min(1) + EPS)
    ay_, by_, py_ = av[..., 1], bv[..., 1], p[..., 1]
    ax_, bx_, px_ = av[..., 0], bv[..., 0], p[..., 0]
    up = (ay_ <= py_) & (py_ < by_)
    dn = (ay_ > py_) & (py_ >= by_)
    left = (bx_-ax_)*(py_-ay_) - (px_-ax_)*(by_-ay_) > 0
    w = np.where(up & left, 1.0, 0.0) + np.where(dn & ~left, -1.0, 0.0)
    wn = w.sum(1)
    sdf = np.where(wn != 0, -dist, dist)
    cov = 1.0/(1.0 + np.exp(sdf/0.01))
    la_all = cov * a_s[:, None, None]
    rgb = np.zeros((height, width, 3))
    ca = np.zeros((height, width, 1))
    for s in range(len(a_s)):
        la = la_all[s][..., None]
        if csg_s[s]:
            ca2 = ca*(1-la)
            rgb = rgb * (ca2 > 0)
            ca = ca2
        else:
            out_a = la + ca*(1-la)
            safe = np.where(out_a > 0, out_a, 1.0)
            rgb = np.where(out_a > 0, (c_s[s]*la + rgb*ca*(1-la))/safe, 0.0)
            ca = out_a
    return np.clip(rgb*ca, 0, 1).astype(np.float32)


# ------------------------------------------------------------------ driver
ALPHA_S = None
COL_S = None
LAST_RESULT = None


def kernel(P, c, alpha, alive, z, csg, width, height):
    global ALPHA_S, COL_S, LAST_RESULT
    width = int(width)
    height = int(height)
    if width != HW or height != HW or np.asarray(csg).any():
        return _numpy_reference(P, c, alpha, alive, z, csg, width, height)

    pre = _precompute(P, c, alpha, alive, z, csg)
    ALPHA_S = [float(v) for v in pre['a_s']]
    COL_S = [[float(v) for v in row] for row in pre['c_s']]

    from concourse.bass_utils import run_bass_kernel_spmd

    nc = _build_program(pre['pe'], pre['pv'], pre['pc'], pre['cols'],
                        pre['maxw'])

    ident = np.eye(128, dtype=np.float32)
    cvals = [EPS, -float(PEN)] + [0.0]*6
    consts = np.broadcast_to(
        np.asarray(cvals, np.float32)[None, :], (128, 8)).copy()

    in_maps = []
    for cc in range(NCORES):
        in_maps.append(dict(w=np.ascontiguousarray(pre['Wcore'][cc]),
                            xfeat=pre['X18'], ck=pre['ckall'][cc],
                            ident=ident, consts=consts))

    trace = bool(int(os.environ.get('DIFFRAST_TRACE', '0')))
    res = run_bass_kernel_spmd(nc, in_maps, core_ids=list(range(NCORES)),
                               trace=trace)
    LAST_RESULT = res

    img = np.empty((HW, HW, 3), np.float32)
    for cc in range(NCORES):
        o = res.results[cc]['out']            # (3, 144, 128)
        o = o.reshape(3, RPC, CB, 128).transpose(1, 2, 3, 0)  # (48,3,128,3)
        img[cc::NCORES] = o.reshape(RPC, HW, 3)
    return img


# revision 26
# speedup vs baseline: 6.0803x; 1.1305x over previous
"""Trainium2 Bass kernel for the soft Bezier rasterizer (nn_DiffRasterizer).

Contract: kernel(**inputs) takes FULL unsharded inputs (as produced by
reference.setup_inputs()) and returns the FULL (384,384,3) float32 image.

Strategy (pixel-spatial sharding, zero cross-core communication):
  * Core c owns image rows c::8 (strided; keeps per-row-index culled
    instruction shapes identical across the SPMD cores while adjacent
    rows share nearly identical cull lists).
  * Every per-(pixel,segment) quantity is a quadratic in px along a row,
    so the host bakes per (row, col-block) weight matrices over the
    feature vector [dx^2, dx, 1] (dx re-centered per 128-col block).
    The weights/features are 3-way bf16 split (6 product terms, K=18)
    so the PE evaluates them in ONE full-rate bf16 pass with fp32 PSUM
    accumulation -- matching fp32 matmul accuracy at ~4x the speed.
  * Per-row culling: distance candidates keep only segments/vertices
    within DTH of the row (sigmoid(-DTH/0.01) ~ 1e-7 -> invisible);
    winding keeps only segments whose y-interval straddles the row.
    Blocks per (row,cb): [R'(8*pe) | E(8*pe) | D2(8*pv) | C2(8*pc)]
      R' = sqrt(1e9)*(t*-0.5);  E = perp dist^2;  D2 = vertex dist^2
      C2 = 1e25 * cross  (sign-preserving, masked rows only)
    seg candidate = E + relu(R'^2 - 0.25e9); min(cand, D2) == min dist^2
    winding: ssum = sum sign(C2); inside <=> ssum != -sum(sigma)
  * Per-tile work: 1 bf16 matmul (2 if wide), ACT {Square, Relu, Copy,
    Sign} (all share one ACT table -> no table reloads), DVE {add,
    row-level min/sum reduces}. sqrt/sigmoid/compare smalls batched
    once at the end over (128,1152). Premultiplied-alpha composite.
"""
import sys
import os
import numpy as np

for _p in ('/opt/trn_rl_repo',):
    if _p not in sys.path and os.path.isdir(_p):
        sys.path.insert(0, _p)

import ml_dtypes

BF16 = ml_dtypes.bfloat16

N = 8            # shapes
S = 30           # polyline samples per shape
HW = 384         # image height == width
EPS = 1e-8
RSC = float(np.sqrt(1e9))   # R' scale
PEN = np.float32(0.25e9)    # relu threshold = 0.25*RSC^2
CSC = 1e25                  # cross scale (sign saturation)
BIGD = 1e6                  # padding distance^2 (far -> cov 0)
DTH = 0.14                  # cull distance (sigmoid(-14) = 8e-7)
NCORES = 8
RPC = HW // NCORES          # rows per core = 48
CB = 3                      # 128-wide col blocks per row
NT = RPC * CB               # pixel tiles per core = 144
NSMALL = RPC * CB * N       # 1152 end-phase elements per partition


# ---------------------------------------------------------------- host math
def _bezier_to_polyline(cp, n_samples=S):
    t_global = np.linspace(0.0, 4.0 - 4.0 / n_samples, n_samples)
    seg = np.clip(np.floor(t_global).astype(np.int64), 0, 3)
    t = t_global - seg
    ti = 1.0 - t
    basis = np.stack([ti**3, 3*ti**2*t, 3*ti*t**2, t**3], axis=-1)
    idx = np.stack([seg*3, seg*3+1, seg*3+2, (seg*3+3) % 12], axis=-1)
    gathered = cp[:, idx, :]
    return np.einsum('sk,mskd->msd', basis, gathered)


def _split3(x):
    xh = x.astype(BF16).astype(np.float64)
    xm = (x - xh).astype(BF16).astype(np.float64)
    xl = (x - xh - xm).astype(BF16).astype(np.float64)
    return xh, xm, xl


# K-stack order: terms (Xh*Wh),(Xh*Wm),(Xm*Wh),(Xh*Wl),(Xm*Wm),(Xl*Wh)
_XTERM = [0, 0, 1, 0, 1, 2]
_WTERM = [0, 1, 0, 2, 1, 0]


def _precompute(P, c, alpha, alive, z, csg):
    P = np.asarray(P, np.float64)
    sig_alive = 1.0 / (1.0 + np.exp(-np.asarray(alive, np.float64)))
    active = sig_alive > 0.1
    eff_alpha = np.where(active, np.asarray(alpha, np.float64), 0.0)
    order = np.argsort(np.asarray(z, np.float64), kind='stable')
    P_s = P[order]
    c_s = np.asarray(c, np.float64)[order]
    a_s = eff_alpha[order]

    poly = _bezier_to_polyline(P_s).astype(np.float32).astype(np.float64)
    a = poly
    b = np.roll(poly, -1, axis=1)
    ax, ay = a[..., 0].ravel(), a[..., 1].ravel()      # (240,) m-major
    bx, by = b[..., 0].ravel(), b[..., 1].ravel()
    abx, aby = bx - ax, by - ay
    ab2e = abx**2 + aby**2 + EPS
    inv = 1.0 / ab2e
    ylo = np.minimum(ay, by)
    yhi = np.maximum(ay, by)

    y = np.linspace(0.0, 1.0, HW)
    x = np.linspace(0.0, 1.0, HW)
    px0 = np.array([x[cb*128:(cb+1)*128].mean() for cb in range(CB)])
    dxf = x - np.repeat(px0, 128)
    xfeat = np.stack([dxf**2, dxf, np.ones_like(dxf)], 0)       # (3,384) f64

    # cull lists per row (global), then shared pads per row-index i
    elists, vlists, clists = [], [], []
    for r in range(HW):
        py = y[r]
        erel = (py > ylo - DTH) & (py < yhi + DTH)
        vrel = np.abs(ay - py) <= DTH
        crel = ((ay <= py) & (py < by)) | ((ay > py) & (py >= by))
        elists.append([np.nonzero(erel.reshape(N, S)[m])[0] for m in range(N)])
        vlists.append([np.nonzero(vrel.reshape(N, S)[m])[0] for m in range(N)])
        clists.append([np.nonzero(crel.reshape(N, S)[m])[0] for m in range(N)])

    pe = np.zeros(RPC, np.int64)
    pv = np.zeros(RPC, np.int64)
    pc = np.zeros(RPC, np.int64)
    for i in range(RPC):
        rows = [i*NCORES + cc for cc in range(NCORES)]
        pe[i] = max(1, max(len(elists[r][m]) for r in rows for m in range(N)))
        pv[i] = max(1, max(len(vlists[r][m]) for r in rows for m in range(N)))
        pc[i] = max(1, max(len(clists[r][m]) for r in rows for m in range(N)))
    cols = 16*pe + 8*pv + 8*pc
    maxw = int(cols.max())

    # per (row, cb): quadratic coefficient rows [A,B,C] for each column,
    # f64, re-centered per col-block; then split3 -> (18, cols) bf16.
    sigma_all = np.zeros((HW, N*S))
    for r in range(HW):
        py = y[r]
        up = (ay <= py) & (py < by)
        dn = (ay > py) & (py >= by)
        sigma_all[r] = np.where(up, 1.0, 0.0) - np.where(dn, 1.0, 0.0)
    ck = (-sigma_all.reshape(HW, N, S).sum(-1)).astype(np.float32)  # (384,8)

    def row_coeffs(r, i):
        """(3, cols_i) f64 coefficient matrix for global row r, index i."""
        py = y[r]
        e = aby*py - abx*ax - aby*ay                    # D1 = abx*px + e
        npe, npv, npc = pe[i], pv[i], pc[i]
        C = np.zeros((3, cols[i]))
        for m in range(N):
            el, vl, cl = elists[r][m], vlists[r][m], clists[r][m]
            base_r = m*npe
            base_e = 8*npe + m*npe
            base_v = 16*npe + m*npv
            base_c = 16*npe + 8*npv + m*npc
            sel = m*S + el
            # R2 = 1e9*(t*-0.5)^2, t*-0.5 = a*px + b  (quadratic in px)
            _a = abx[sel]*inv[sel]
            _b = e[sel]*inv[sel] - 0.5
            C[0, base_r:base_r+len(el)] = (RSC*RSC)*_a*_a
            C[1, base_r:base_r+len(el)] = (RSC*RSC)*2.0*_a*_b
            C[2, base_r:base_r+len(el)] = (RSC*RSC)*_b*_b
            # E = D2 - D1^2*inv
            C[0, base_e:base_e+len(el)] = 1.0 - abx[sel]**2*inv[sel]
            C[1, base_e:base_e+len(el)] = -2*ax[sel] - 2*abx[sel]*e[sel]*inv[sel]
            C[2, base_e:base_e+len(el)] = (ax[sel]**2 + (py - ay[sel])**2
                                           - e[sel]**2*inv[sel])
            C[2, base_e+len(el):base_e+npe] = BIGD      # pads
            if len(el) < npe:
                C[2, base_r+len(el):base_r+npe] = 0.0
            # D2 = px^2 - 2 ax px + ax^2 + (py-ay)^2
            sv = m*S + vl
            C[0, base_v:base_v+len(vl)] = 1.0
            C[1, base_v:base_v+len(vl)] = -2*ax[sv]
            C[2, base_v:base_v+len(vl)] = ax[sv]**2 + (py - ay[sv])**2
            C[2, base_v+len(vl):base_v+npv] = BIGD
            # C2 = CSC * cross ;  cross = -aby*px + (abx*(py-ay) + ax*aby)
            sc = m*S + cl
            C[0, base_c:base_c+len(cl)] = 0.0
            C[1, base_c:base_c+len(cl)] = -aby[sc]*CSC
            C[2, base_c:base_c+len(cl)] = (abx[sc]*(py - ay[sc])
                                           + ax[sc]*aby[sc])*CSC
        return C

    # Re-center per col-block and split
    Wcore = np.zeros((NCORES, RPC, CB, 18, maxw), BF16)
    for i in range(RPC):
        for cc in range(NCORES):
            r = i*NCORES + cc
            Cq = row_coeffs(r, i)                       # (3, cols_i)
            A, B_, C0 = Cq[0], Cq[1], Cq[2]
            for cb in range(CB):
                p0 = px0[cb]
                W = np.stack([A, 2*A*p0 + B_, A*p0*p0 + B_*p0 + C0], 0)
                Wh, Wm, Wl = _split3(W)
                Wparts = (Wh, Wm, Wl)
                for t6 in range(6):
                    Wcore[cc, i, cb, t6*3:(t6+1)*3, :cols[i]] = \
                        Wparts[_WTERM[t6]].astype(BF16)

    Xh, Xm, Xl = _split3(xfeat)
    Xparts = (Xh, Xm, Xl)
    X18 = np.zeros((18, CB, 128), BF16)
    for cb in range(CB):
        for t6 in range(6):
            X18[t6*3:(t6+1)*3, cb, :] = \
                Xparts[_XTERM[t6]][:, cb*128:(cb+1)*128].astype(BF16)

    # ckall per core: (128, i, cb, s) replicated partitions
    ckall = np.zeros((NCORES, 128, RPC, CB, N), np.float32)
    for cc in range(NCORES):
        for i in range(RPC):
            ckall[cc, :, i, :, :] = ck[i*NCORES + cc][None, None, :]

    Wcore = np.ascontiguousarray(Wcore.transpose(0, 1, 3, 2, 4))
    return dict(Wcore=Wcore,   # (NCORES, RPC, 18, CB, maxw)
                X18=X18, ckall=ckall.reshape(NCORES, 128, NSMALL),
                pe=pe, pv=pv, pc=pc, cols=cols, maxw=maxw,
                c_s=c_s.astype(np.float32), a_s=a_s.astype(np.float32),
                poly=poly.astype(np.float32))


# ------------------------------------------------------------- bass program
def _build_program(pe, pv, pc, cols, maxw):
    import concourse.bass as bass
    import concourse.bacc as bacc
    import concourse.mybir as mybir
    from concourse import tile

    dt = mybir.dt.float32
    bt = mybir.dt.bfloat16
    AF = mybir.ActivationFunctionType
    ALU = mybir.AluOpType
    AX = mybir.AxisListType

    nc = bacc.Bacc()
    w_d = nc.declare_dram_parameter("w", [RPC, 18, CB, maxw], bt,
                                    isOutput=False)
    xf_d = nc.declare_dram_parameter("xfeat", [18, CB, 128], bt,
                                     isOutput=False)
    ck_d = nc.declare_dram_parameter("ck", [128, NSMALL], dt, isOutput=False)
    ident_d = nc.declare_dram_parameter("ident", [128, 128], dt,
                                        isOutput=False)
    cst_d = nc.declare_dram_parameter("consts", [128, 8], dt, isOutput=False)
    out_d = nc.declare_dram_parameter("out", [3, NT, 128], dt, isOutput=True)

    with tile.TileContext(nc) as tc:
        with (
            tc.tile_pool(name="const", bufs=1) as cpool,
            tc.tile_pool(name="wpool", bufs=3) as wpool,
            tc.tile_pool(name="work", bufs=4) as work,
            tc.tile_pool(name="slabs", bufs=1) as slabs,
            tc.tile_pool(name="psA", bufs=4, space=bass.MemorySpace.PSUM) as psa,
            tc.tile_pool(name="psB", bufs=2, space=bass.MemorySpace.PSUM) as psb,
            tc.tile_pool(name="psT", bufs=1, space=bass.MemorySpace.PSUM) as pst,
        ):
            xfeat = cpool.tile([18, CB, 128], bt)
            nc.sync.dma_start(xfeat[:], xf_d[:])
            ckt = cpool.tile([128, NSMALL], dt)
            nc.sync.dma_start(ckt[:], ck_d[:])
            ident = cpool.tile([128, 128], dt)
            nc.sync.dma_start(ident[:], ident_d[:])
            cst = cpool.tile([128, 8], dt)
            nc.sync.dma_start(cst[:], cst_d[:])
            c_eps = cst[:, 0:1]

            la_all = slabs.tile([128, N, NT], dt)
            mindall = slabs.tile([128, NSMALL], dt)
            ssumall = slabs.tile([128, NSMALL], dt)

            for i in range(RPC):
                npe, npv, npc = int(pe[i]), int(pv[i]), int(pc[i])
                ci = int(cols[i])
                wt = wpool.tile([18, CB, maxw], bt, tag="w")
                nc.sync.dma_start(wt[:], w_d[i])
                slab = work.tile([128, CB, N, npe + npv], dt, tag="slab")
                s1 = work.tile([128, CB, N, npc], dt, tag="s1")
                for cb in range(CB):
                    if ci <= 512:
                        pA = psa.tile([128, ci], dt, tag="pA")
                        nc.tensor.matmul(pA[:], xfeat[:, cb, :], wt[:, cb, 0:ci],
                                         start=True, stop=True)
                        ap_R = pA[:, 0:8*npe]
                        ap_E = pA[:, 8*npe:16*npe]
                        ap_D = pA[:, 16*npe:16*npe+8*npv]
                        ap_C = pA[:, 16*npe+8*npv:ci]
                    else:
                        pA = psa.tile([128, 16*npe], dt, tag="pA")
                        pB = psb.tile([128, ci - 16*npe], dt, tag="pB")
                        nc.tensor.matmul(pA[:], xfeat[:, cb, :],
                                         wt[:, cb, 0:16*npe],
                                         start=True, stop=True)
                        nc.tensor.matmul(pB[:], xfeat[:, cb, :],
                                         wt[:, cb, 16*npe:ci],
                                         start=True, stop=True)
                        ap_R = pA[:, 0:8*npe]
                        ap_E = pA[:, 8*npe:16*npe]
                        ap_D = pB[:, 0:8*npv]
                        ap_C = pB[:, 8*npv:8*npv+8*npc]

                    pen = work.tile([128, 8*npe], dt, tag="pen")
                    nc.scalar.activation(pen[:], ap_R, AF.Relu,
                                         bias=cst[:, 1:2])
                    # seg candidates -> slab[..., 0:npe]
                    nc.vector.tensor_tensor(
                        slab[:, cb, :, 0:npe], ap_E, pen[:], ALU.add)
                    # vertex candidates -> slab[..., npe:npe+npv]
                    nc.scalar.activation(slab[:, cb, :, npe:npe+npv],
                                         ap_D, AF.Copy)
                    nc.scalar.activation(s1[:, cb], ap_C, AF.Sign)

                # row-level reduces straight into the end-phase slabs
                nc.vector.tensor_reduce(
                    mindall[:, i*CB*N:(i+1)*CB*N], slab[:], AX.X, ALU.min)
                nc.vector.tensor_reduce(
                    ssumall[:, i*CB*N:(i+1)*CB*N], s1[:], AX.X, ALU.add)

            # ---- end phase, batched over (128, 1152)
            m0 = slabs.tile([128, NSMALL], dt)
            nc.vector.tensor_scalar_max(m0[:], mindall[:], 0.0)
            sd = slabs.tile([128, NSMALL], dt)
            nc.scalar.activation(sd[:], m0[:], AF.Sqrt, bias=c_eps)
            ins = slabs.tile([128, NSMALL], dt)
            nc.vector.tensor_tensor(ins[:], ssumall[:], ckt[:], ALU.not_equal)
            sgn = slabs.tile([128, NSMALL], dt)
            nc.vector.tensor_scalar(sgn[:], ins[:], -2.0, 1.0,
                                    ALU.mult, ALU.add)
            sdf = slabs.tile([128, NSMALL], dt)
            nc.vector.tensor_tensor(sdf[:], sgn[:], sd[:], ALU.mult)
            # cov = sigmoid(-100*sdf); source order (i,cb,s) -> la_all[s, t]
            la_t = la_all[:].rearrange("p n (i cb) -> p i cb n", cb=CB)
            nc.scalar.activation(la_t, sdf[:], AF.Sigmoid, scale=-100.0)

            # ---- composite: prgb' = prgb + (alpha_s*cov)*(col_ch - prgb)
            prgb = slabs.tile([128, 3, NT], dt)
            nc.vector.memset(prgb[:], 0.0)
            for s in range(N):
                la_s = la_all[:, s, :]
                for ch in range(3):
                    diff = work.tile([128, NT], dt, tag="diff")
                    nc.vector.tensor_scalar(diff[:], prgb[:, ch, :],
                                            float(-ALPHA_S[s]),
                                            float(ALPHA_S[s] * COL_S[s][ch]),
                                            ALU.mult, ALU.add)
                    m = work.tile([128, NT], dt, tag="m")
                    nc.vector.tensor_tensor(m[:], la_s, diff[:], ALU.mult)
                    nc.vector.tensor_tensor(prgb[:, ch, :], prgb[:, ch, :],
                                            m[:], ALU.add)
            for ch in range(3):
                nc.vector.tensor_scalar(prgb[:, ch, :], prgb[:, ch, :],
                                        0.0, 1.0, ALU.max, ALU.min)

            # ---- output transpose: (128 p, 144 t) -> (144 t, 128 p) per ch
            for ch in range(3):
                t1 = pst.tile([128, 128], dt, tag="t1")
                nc.tensor.transpose(t1[:], prgb[:, ch, 0:128], ident[:])
                o1 = work.tile([128, 128], dt, tag="o1")
                nc.vector.tensor_copy(o1[:], t1[:])
                nc.sync.dma_start(out_d[ch, 0:128, :], o1[:])
                t2 = pst.tile([16, 128], dt, tag="t2")
                nc.tensor.transpose(t2[:], prgb[:, ch, 128:NT], ident[:])
                o2 = work.tile([16, 128], dt, tag="o2")
                nc.vector.tensor_copy(o2[:], t2[:])
                nc.sync.dma_start(out_d[ch, 128:NT, :], o2[:])

    nc.compile()
    return nc


# ---------------------------------------------------------------- fallback
def _numpy_reference(P, c, alpha, alive, z, csg, width, height):
    """Direct numpy port of reference.py (csg-capable); slow but exact."""
    P = np.asarray(P, np.float32)
    sig = 1.0 / (1.0 + np.exp(-np.asarray(alive, np.float64)))
    eff_alpha = np.where(sig > 0.1, np.asarray(alpha, np.float64), 0.0)
    order = np.argsort(np.asarray(z, np.float64), kind='stable')
    P_s, c_s = P[order], np.asarray(c, np.float64)[order]
    a_s, csg_s = eff_alpha[order], np.asarray(csg, bool)[order]
    poly = _bezier_to_polyline(P_s.astype(np.float64))
    a = poly
    b = np.roll(poly, -1, axis=1)
    y = np.linspace(0, 1, height)
    x = np.linspace(0, 1, width)
    gx, gy = np.meshgrid(x, y)
    p = np.stack([gx, gy], -1)[None, None]
    av = a[:, :, None, None, :]
    bv = b[:, :, None, None, :]
    ab = bv - av
    ap = p - av
    t = np.clip((ap*ab).sum(-1) / ((ab*ab).sum(-1) + EPS), 0, 1)
    diff = p - (av + t[..., None]*ab)
    dist = np.sqrt((diff*diff).sum(-1).min(1) + EPS)
    ay_, by_, py_ = av[..., 1], bv[..., 1], p[..., 1]
    ax_, bx_, px_ = av[..., 0], bv[..., 0], p[..., 0]
    up = (ay_ <= py_) & (py_ < by_)
    dn = (ay_ > py_) & (py_ >= by_)
    left = (bx_-ax_)*(py_-ay_) - (px_-ax_)*(by_-ay_) > 0
    w = np.where(up & left, 1.0, 0.0) + np.where(dn & ~left, -1.0, 0.0)
    wn = w.sum(1)
    sdf = np.where(wn != 0, -dist, dist)
    cov = 1.0/(1.0 + np.exp(sdf/0.01))
    la_all = cov * a_s[:, None, None]
    rgb = np.zeros((height, width, 3))
    ca = np.zeros((height, width, 1))
    for s in range(len(a_s)):
        la = la_all[s][..., None]
        if csg_s[s]:
            ca2 = ca*(1-la)
            rgb = rgb * (ca2 > 0)
            ca = ca2
        else:
            out_a = la + ca*(1-la)
            safe = np.where(out_a > 0, out_a, 1.0)
            rgb = np.where(out_a > 0, (c_s[s]*la + rgb*ca*(1-la))/safe, 0.0)
            ca = out_a
    return np.clip(rgb*ca, 0, 1).astype(np.float32)


# ------------------------------------------------------------------ driver
ALPHA_S = None
COL_S = None
LAST_RESULT = None


def kernel(P, c, alpha, alive, z, csg, width, height):
    global ALPHA_S, COL_S, LAST_RESULT
    width = int(width)
    height = int(height)
    if width != HW or height != HW or np.asarray(csg).any():
        return _numpy_reference(P, c, alpha, alive, z, csg, width, height)

    pre = _precompute(P, c, alpha, alive, z, csg)
    ALPHA_S = [float(v) for v in pre['a_s']]
    COL_S = [[float(v) for v in row] for row in pre['c_s']]

    from concourse.bass_utils import run_bass_kernel_spmd

    nc = _build_program(pre['pe'], pre['pv'], pre['pc'], pre['cols'],
                        pre['maxw'])

    ident = np.eye(128, dtype=np.float32)
    cvals = [EPS, -float(PEN)] + [0.0]*6
    consts = np.broadcast_to(
        np.asarray(cvals, np.float32)[None, :], (128, 8)).copy()

    in_maps = []
    for cc in range(NCORES):
        in_maps.append(dict(w=np.ascontiguousarray(pre['Wcore'][cc]),
                            xfeat=pre['X18'], ck=pre['ckall'][cc],
                            ident=ident, consts=consts))

    trace = bool(int(os.environ.get('DIFFRAST_TRACE', '0')))
    res = run_bass_kernel_spmd(nc, in_maps, core_ids=list(range(NCORES)),
                               trace=trace)
    LAST_RESULT = res

    img = np.empty((HW, HW, 3), np.float32)
    for cc in range(NCORES):
        o = res.results[cc]['out']            # (3, 144, 128)
        o = o.reshape(3, RPC, CB, 128).transpose(1, 2, 3, 0)  # (48,3,128,3)
        img[cc::NCORES] = o.reshape(RPC, HW, 3)
    return img
